# revision 1
# baseline (speedup 1.0000x reference)
"""Trainium2 Bass kernel for nn_AttentionLayer_10591389352529 (sparse window attention).

Reference computation (per batch b, query n):
    local[n,k] = feat gathered at 3x3x3 window around proj_coord[n]   (k=0..26, c=128)
    q[n]       = q_feat[n] @ q_w.T + q_b
    score[n,k] = q[n] . (k_w @ local[n,k] + k_b)
    a[n,:]     = softmax(score[n,:])
    out[n]     = q_feat[n] + sum_k a[n,k] * (v_w @ local[n,k] + v_b)

Algebraic reformulation used here (cuts ~25x the flops):
    score[n,k] = qk[n] . local[n,k] + sb[n]
        where [qk | sb][n] = q_feat[n] @ [q_w.T@k_w | q_w.T@k_b] + [q_b@k_w | q_b.k_b]
    out[n]     = q_feat[n] + v_w @ (sum_k a[n,k] local[n,k]) + v_b     (since sum_k a = 1)

Sharding: flat (B*N)=16384 query axis split across 8 cores (2048 queries each);
each core receives only its batch's feat volume, stored channels-last with the W
axis edge-padded by 1 so each (dd,hh) window row is one contiguous 3*128-float
chunk. The gather runs on-device via indirect DMA with host-precomputed voxel
indices (host does only O(N*9) integer index math + a layout transpose).

Walrus sync-wait limits shape the structure: a Matmult takes only ONE
input-operand wait (all input waits land on the LDWEIGHTS struct) and a DMACopy
takes two. Hence: PE "warmup" matmuls absorb every const-tensor dependency into
PE's vector clock; q_feat (both layouts) is preloaded to SBUF whole; one
indirect gather covers two query tiles (8 gathers over 8 SWDGE lanes -> no
lane-reuse waits); outputs batch through an SBUF staging buffer.
"""

import math
from contextlib import ExitStack

import numpy as np

import concourse.bass as bass
import concourse.tile as tile
from concourse import mybir
from concourse._compat import with_exitstack
from concourse.dve_ops import AFFINE_THEN_ADD, TENSOR_TENSOR_REDUCE
from concourse.tile_rust import add_dep_helper
import concourse.tile_sem_assignment as _tsa

# Single SWDGE completion sem so the kernel-tail Drain's wait list (one per
# touched semaphore) fits the walrus CTRL_NO struct, and so every SWDGE DMA's
# slot-WAW/own-lane/RAW waits merge onto ONE semaphore (one wait slot).
_tsa.NUM_SWDGE_GLOBAL_SEMS = 1

FP32 = mybir.dt.float32
INT32 = mybir.dt.int32
P = 128

B, N, C = 2, 8192, 128
D = H = W = 48
WP = W + 2  # W axis edge-padded by 1 on each side
NVOX = D * H * WP  # rows of the channels-last volume, per batch
NCORES = 8
QPC = B * N // NCORES  # queries per core
KWIN = 27  # 3x3x3 window
KW = KWIN * P  # gathered row length per tile (3456)

# how many of the 27 weighted-sum scaled-copies run on the scalar (ACT) engine;
# the rest run on the vector (DVE) engine. Chosen to balance the two engines'
# per-tile spans (DVE: 27 fused score-reduces + leftovers; ACT: exp + copies).
N_ACT_MAC = 19


@with_exitstack
def attention_body(
    ctx: ExitStack,
    tc: tile.TileContext,
    *,
    featcl: bass.AP,  # (nvox, 128) f32 channels-last padded volume
    qpack: bass.AP,  # (128, 2*qpc + ngath*18) f32 [qft | qf by-tile | gidx bits]
    wpack: bass.AP,  # (128, 257) f32 [q_w.T@k_w | q_w.T@k_b | v_w.T]
    bpack: bass.AP,  # (1, 257) f32   [q_b@k_w | q_b.k_b | v_b]
    out: bass.AP,  # (qpc, 128) f32
    n_act_mac: int = N_ACT_MAC,
):
    nc = tc.nc
    qpc = out.shape[0]
    ntiles = qpc // P
    assert qpc % (2 * P) == 0
    ngath = ntiles // 2
    GBUFS = 4
    # single output DMA at the end: a second DMA writing the same `out` tensor
    # would carry a tensor-level WAW wait on top of its data wait (2 > the
    # 1-wait DMA budget)
    ogroup = ntiles

    const = ctx.enter_context(tc.tile_pool(name="const", bufs=1))
    gath_pool = ctx.enter_context(tc.tile_pool(name="gath", bufs=GBUFS))
    qk_pool = ctx.enter_context(tc.tile_pool(name="qk", bufs=3))
    sc_pool = ctx.enter_context(tc.tile_pool(name="sc", bufs=16))
    small_pool = ctx.enter_context(tc.tile_pool(name="small", bufs=16))
    scratch_pool = ctx.enter_context(tc.tile_pool(name="scratch", bufs=4))
    # >= 28 so scaled-tile slots are never reused within a tile: the WAR wait
    # against PE would otherwise exceed the 1-wait-slot instruction limit
    mac_pool = ctx.enter_context(tc.tile_pool(name="mac", bufs=32))
    m_pool = ctx.enter_context(tc.tile_pool(name="m", bufs=4))
    stage_pool = ctx.enter_context(tc.tile_pool(name="stage", bufs=1))
    psum = ctx.enter_context(tc.tile_pool(name="psum", bufs=2, space="PSUM"))

    # constants / weights / whole q_feat in THREE packed HWDGE DMAs (fewer
    # DMAHW semaphores -> shorter kernel-tail drain wait list):
    #   wpack: [wqk (129) | vwt (128)]            (128, 257)
    #   bpack: [bqk (129) | vb (128)]             (1, 257)
    #   qpack: [qft | qf by-tile | gidx bits]     (128, 2*qpc + ngath*18)
    # const loads ride SWDGE (gpsimd) too: no HWDGE semaphore is ever touched
    qpack_sb = const.tile([P, 2 * qpc + ngath * 18], FP32)
    nc.gpsimd.dma_start(qpack_sb[:], qpack[:, :])
    wpack_sb = const.tile([P, 257], FP32)
    nc.gpsimd.dma_start(wpack_sb[:], wpack[:, :])
    bpack_sb = const.tile([1, 257], FP32)
    nc.gpsimd.dma_start(bpack_sb[:], bpack[:, :])
    wqk_sb = wpack_sb[:, 0:129]
    vwt_sb = wpack_sb[:, 129:257]
    bqk_sb = bpack_sb[:, 0:129]
    vb_sb = bpack_sb[:, 129:257]
    qft_full = qpack_sb[:, 0:qpc]
    qf_full = qpack_sb[:, qpc : 2 * qpc]
    gidx_sb = qpack_sb[:, 2 * qpc :].bitcast(INT32)

    ones_sb = const.tile([1, P], FP32)
    nc.vector.memset(ones_sb[:], 1.0)
    ident = const.tile([P, P], FP32)
    from concourse.masks import make_identity

    make_identity(nc, ident[:])

    # rotating dummy targets for clock-absorbing "touch" copies: rotation keeps
    # each touch's WAW dep old enough to be already-observed (0 extra waits)
    junk_pool = ctx.enter_context(tc.tile_pool(name="junk", bufs=8))

    # PE warmup: walrus's Matmult lowering supports a single input-side
    # sync-wait slot, so absorb every const-tensor dependency into PE's vector
    # clock up front with 1-column matmuls (each waits on one thing only).
    wu = psum.tile([P, 129], FP32, space="PSUM", tag="qk_ps")
    nc.tensor.matmul(wu[:, 0:1], lhsT=ident[:], rhs=ident[:, 0:1], start=True, stop=True)
    for cst in (wpack_sb[:], bpack_sb[:], qpack_sb[:], ones_sb[:]):
        nc.tensor.matmul(
            wu[0:1, 0:1], lhsT=cst[:, 0:1], rhs=cst[:, 0:1], start=True, stop=True
        )

    stage = None
    sig_hist = []
    for t in range(ntiles):
        ts = bass.ts(t, P)
        g, tt = divmod(t, 2)

        if tt == 0:
            # before reusing a gather slot, absorb the previous users' DVE/ACT
            # ticks into the Pool clock via tiny gpsimd touches, so the
            # indirect DMA itself stays within its 2 wait slots
            touches = []
            if g >= GBUFS:
                # tiny SWDGE DMAs reading the reused slot's end-of-use signal
                # cells; SBUF->DRAM (2 wait slots) since each carries one
                # reader-done wait plus possibly its own-lane FIFO wait. The
                # gather itself then only needs its slot WAW wait.
                sg_prev = sig_hist[g - GBUFS]
                jtd = junk_pool.tile([1, 1], FP32, tag="junk_touch_d")
                touches.append(nc.gpsimd.tensor_copy(jtd[:], sg_prev[0:1, 0:1]))
                jta = junk_pool.tile([1, 1], FP32, tag="junk_touch_a")
                touches.append(nc.gpsimd.tensor_copy(jta[:], sg_prev[0:1, 1:2]))
            # gather for tiles 2g and 2g+1: 18 chunks x (3 vox * 128 ch), one
            # indirect DMA per chunk index j (the HW DGE consumes exactly ONE
            # index per partition and streams the whole dest row from it).
            # Separate tiles per j avoid intra-period WAW serialization.
            # j < 9 -> tile 2g, j >= 9 -> tile 2g+1.
            gath = [
                gath_pool.tile([P, 3 * P], FP32, tag=f"gath{j}", name=f"gath{j}_{g}")
                for j in range(18)
            ]
            for j in range(18):
                gi = nc.gpsimd.indirect_dma_start(
                    out=gath[j][:],
                    out_offset=None,
                    in_=featcl[:, :],
                    in_offset=bass.IndirectOffsetOnAxis(
                        ap=gidx_sb[:, g * 18 + j : g * 18 + j + 1], axis=0
                    ),
                )
                # keep the scheduler from hoisting the gather above the
                # touches that pre-absorb its WAR ticks
                for tch in touches:
                    add_dep_helper(
                        gi.ins, tch.ins, sync=False, reason="gather after touches"
                    )

        def slab(k):
            j, v = tt * 9 + k // 3, k % 3
            return gath[j][:, v * P : (v + 1) * P]

        # ---- qk = q_feat @ [W1|w2] + [c1|c2]  (PE) ----
        qk_ps = psum.tile([P, 129], FP32, space="PSUM", tag="qk_ps")
        nc.tensor.matmul(
            qk_ps[:], lhsT=qft_full[:, ts], rhs=wqk_sb[:], start=True, stop=False
        )
        nc.tensor.matmul(
            qk_ps[:], lhsT=ones_sb[:], rhs=bqk_sb[:], start=False, stop=True
        )
        # ACT is nearer PSUM; the cross-engine wait this puts on the first
        # score op is legalized into an EventSemaphore by _legalize_waits.
        qk_sb = qk_pool.tile([P, 129], FP32)
        nc.scalar.copy(qk_sb[:], qk_ps[:])

        # ---- scores[n,k] = qk[n] . slab_k[n] + sb[n]  (DVE, fused custom op:
        #      accum_out = s0 + sum(in0*in1*s1); the plain TENSOR_TENSOR_REDUCE
        #      ISA opcode crashes this runtime) ----
        scores = sc_pool.tile([P, KWIN], FP32, tag="scores")
        for k in range(KWIN):
            scr = scratch_pool.tile([P, P], FP32, tag="ttr_scr")
            nc.vector._custom_dve(
                TENSOR_TENSOR_REDUCE,
                out=scr[:],
                in0=qk_sb[:, 0:P],
                in1=slab(k),
                s0=qk_sb[:, P : P + 1],
                s1=1.0,
                accum_out=scores[:, k : k + 1],
            )

        # ---- softmax pieces (max-subtraction required: the ACT exp spline
        # returns non-finite values outside its fitted range on HW) ----
        negmax = small_pool.tile([P, 1], FP32, tag="negmax")
        nc.vector.tensor_reduce(
            out=negmax[:],
            in_=scores[:],
            axis=mybir.AxisListType.X,
            op=mybir.AluOpType.max,
            negate=True,
        )
        e = sc_pool.tile([P, KWIN], FP32, tag="e")
        sumexp = small_pool.tile([P, 1], FP32, tag="sumexp")
        nc.scalar.activation(
            e[:],
            scores[:],
            mybir.ActivationFunctionType.Exp,
            bias=negmax[:],
            scale=1.0,
            accum_out=sumexp[:],
        )
        recip = small_pool.tile([P, 1], FP32, tag="recip")
        nc.vector.reciprocal(recip[:], sumexp[:])
        # absorb recip (DVE) into ACT's clock so the mr_sb scaled-copy below
        # only needs its PE wait
        jr = junk_pool.tile([1, 1], FP32, tag="junk_recip")
        nc.scalar.copy(jr[:], recip[0:1, :])

        # ---- m = sum_k e[:,k] * slab_k, accumulated in PSUM via identity
        #      matmuls; the 1/sumexp normalization is folded into the
        #      PSUM->SBUF copy ----
        m_ps = psum.tile([P, P], FP32, space="PSUM", tag="m_ps")
        for k in range(n_act_mac):
            scaled = mac_pool.tile([P, P], FP32, tag="scaled")
            nc.scalar.mul(scaled[:], slab(k), e[:, k : k + 1])
            last_act_scaled = scaled
            nc.tensor.matmul(
                m_ps[:], lhsT=ident[:], rhs=scaled[:], start=(k == 0), stop=False
            )
        # DVE's share accumulates in SBUF via a fused multiply-add chain
        # (acc = slab*e_k + acc) -> one PE merge matmul instead of 7
        acc = mac_pool.tile([P, P], FP32, tag="dveacc")
        nc.vector.tensor_scalar_mul(acc[:], slab(n_act_mac), e[:, n_act_mac : n_act_mac + 1])
        for k in range(n_act_mac + 1, KWIN):
            acc2 = mac_pool.tile([P, P], FP32, tag="dveacc")
            nc.vector._custom_dve(
                AFFINE_THEN_ADD,
                out=acc2[:],
                in0=slab(k),
                in1=acc[:],
                s0=e[:, k : k + 1],
                s1=0.0,
            )
            acc = acc2
        last_dve_scaled = acc
        nc.tensor.matmul(m_ps[:], lhsT=ident[:], rhs=acc[:], start=False, stop=True)
        mr_sb = m_pool.tile([P, P], FP32, tag="mr_sb")
        nc.scalar.mul(mr_sb[:], m_ps[:], recip[:])

        # ---- x = mr @ v_w.T + q_feat + v_b  (PE; transpose mr first) ----
        mt_ps = psum.tile([P, P], FP32, space="PSUM", tag="mt_ps")
        nc.tensor.transpose(out=mt_ps[:], in_=mr_sb[:], identity=ident[:])
        mt_sb = m_pool.tile([P, P], FP32, tag="mt_sb")
        nc.vector.tensor_copy(mt_sb[:], mt_ps[:])

        x_ps = psum.tile([P, P], FP32, space="PSUM", tag="x_ps")
        nc.tensor.matmul(x_ps[:], lhsT=mt_sb[:], rhs=vwt_sb[:], start=True, stop=False)
        nc.tensor.matmul(
            x_ps[:],
            lhsT=ident[:],
            rhs=qf_full[:, t * P : (t + 1) * P],
            start=False,
            stop=False,
        )
        nc.tensor.matmul(x_ps[:], lhsT=ones_sb[:], rhs=vb_sb[:], start=False, stop=True)

        # ---- stage output; one SWDGE DMA at the end. A gpsimd touch of the
        # staging buffer first absorbs the ACT copies' ticks into the SWDGE
        # stream so the out DMA needs only its own-sem wait ----
        if t % ogroup == 0:
            stage = stage_pool.tile([P, ogroup * P], FP32, tag="stage")
        nc.scalar.copy(stage[:, bass.ts(t % ogroup, P)], x_ps[:])
        if t % ogroup == ogroup - 1:
            t0 = t - (ogroup - 1)
            jout = junk_pool.tile([1, 1], FP32, tag="junk_out")
            otch = nc.gpsimd.tensor_copy(jout[:], stage[0:1, ogroup * P - 1 :])
            od = nc.gpsimd.dma_start(
                out.rearrange("(t p) c -> p t c", p=P)[:, t0 : t0 + ogroup, :],
                stage[:].rearrange("p (t c) -> p t c", c=P),
            )
            add_dep_helper(od.ins, otch.ins, sync=False, reason="out after stage touch")

        if tt == 1:
            # end-of-gather-period signals: each reads the LAST scaled tile
            # its engine produced this tile, so the write is necessarily
            # scheduled after that engine's final `gath` read; a touch of this
            # tile before the slot-reusing gather then implies all prior
            # readers are done
            sg = junk_pool.tile([1, 2], FP32, tag="sig")
            nc.vector.tensor_copy(sg[0:1, 0:1], last_dve_scaled[0:1, 0:1])
            nc.scalar.copy(sg[0:1, 1:2], last_act_scaled[0:1, 0:1])
            sig_hist.append(sg)


def build_program(qpc: int, nvox: int, n_act_mac: int = N_ACT_MAC):
    """Build the SPMD Bass program. Returns nc."""
    nc = bass.Bass("TRN2", target_bir_lowering=False, debug=False, num_devices=NCORES)
    ngath = qpc // (2 * P)
    aps = {}
    decl = [
        ("featcl", (nvox, C), FP32, False),
        ("qpack", (C, 2 * qpc + ngath * 18), FP32, False),
        ("wpack", (C, 257), FP32, False),
        ("bpack", (1, 257), FP32, False),
        ("out", (qpc, C), FP32, True),
    ]
    for name, shape, dt, is_out in decl:
        kind = "ExternalOutput" if is_out else "ExternalInput"
        aps[name] = nc.dram_tensor(name, list(shape), dt, kind=kind).ap()
    with tile.TileContext(nc) as tc:
        attention_body(tc, n_act_mac=n_act_mac, **aps)
    # populate .instr bytes for InstISA subclasses (TensorTensorReduce);
    # Bacc.compile() does this but the raw-Bass path does not.
    mybir.codegen_inst_isa_subclasses(nc)
    _legalize_waits(nc)
    return nc


def _legalize_waits(nc, max_waits: int = 1):
    """This walrus build accepts only ONE sync-wait slot per instruction
    struct. For any instruction Tile scheduled with more waits, keep the last
    and hoist the rest onto preceding same-engine EventSemaphore instructions
    (the engine queue is FIFO, so waiting before the instruction is
    equivalent to waiting on it)."""
    for f in nc.m.functions:
        for blk in f.blocks:
            insts = blk.instructions
            i = 0
            while i < len(insts):
                inst = insts[i]
                si = inst.sync_info
                if si is not None and len(si.on_wait) > max_waits:
                    waits = list(si.on_wait)
                    pre = []
                    while len(waits) > max_waits:
                        chunk, waits = waits[:max_waits], waits[max_waits:]
                        pre.append(
                            mybir.InstEventSemaphore(
                                name=f"{inst.name}-ws{len(pre)}",
                                engine=inst.engine,
                                ins=[],
                                outs=[],
                                bass_nofuse=True,
                                sync_info=mybir.SyncInfo(on_wait=chunk, on_update=[]),
                            )
                        )
                    si.on_wait = waits
                    insts[i:i] = pre
                    i += len(pre)
                i += 1


def pack_queries(qf_c: np.ndarray, gidx_c: np.ndarray) -> np.ndarray:
    """Build the (128, 2*qpc + ngath*18) qpack host tensor:
    [ qft | qf by-tile (p, t, c) | gather chunk indices (p, g, tt*9+j) ]."""
    qpc = qf_c.shape[0]
    ntiles = qpc // P
    ngath = ntiles // 2
    qft = qf_c.T  # (128, qpc)
    qf_bytile = qf_c.reshape(ntiles, P, C).transpose(1, 0, 2).reshape(P, qpc)
    g = gidx_c.reshape(ngath, 2, P, 9)  # (g, tt, p, j)
    g = np.transpose(g, (2, 0, 1, 3)).reshape(P, ngath * 18)  # (p, g*18 + tt*9 + j)
    return np.ascontiguousarray(
        np.concatenate([qft, qf_bytile, g.view(np.float32)], axis=1, dtype=np.float32)
    )


def host_prepare(q_feat, feat, proj_coord, q_w, q_b, k_w, k_b, v_w, v_b):
    """All host-side input marshalling. Returns per-core input maps."""
    q_feat = np.asarray(q_feat, dtype=np.float32)
    feat = np.asarray(feat, dtype=np.float32)
    proj_coord = np.asarray(proj_coord, dtype=np.int32)
    q_w, q_b, k_w, k_b, v_w, v_b = (
        np.asarray(a, dtype=np.float32) for a in (q_w, q_b, k_w, k_b, v_w, v_b)
    )

    # channels-last volume with W edge-padded by 1: (B, D, H, WP, C)
    fcl = np.transpose(feat, (0, 2, 3, 4, 1))  # (B,D,H,W,C)
    fcl = np.pad(fcl, ((0, 0), (0, 0), (0, 0), (1, 1), (0, 0)), mode="edge")
    fcl = np.ascontiguousarray(fcl.reshape(B, NVOX, C))

    # voxel row index of the first (w-1) voxel of each (dd,hh) chunk
    d = proj_coord[..., 0].astype(np.int64)
    h = proj_coord[..., 1].astype(np.int64)
    w = proj_coord[..., 2].astype(np.int64)
    offs = [(dd, hh) for dd in (-1, 0, 1) for hh in (-1, 0, 1)]
    gidx = np.empty((B, N, 9), dtype=np.int32)
    for j, (dd, hh) in enumerate(offs):
        dc = np.clip(d + dd, 0, D - 1)
        hc = np.clip(h + hh, 0, H - 1)
        gidx[..., j] = ((dc * H + hc) * WP + w).astype(np.int32)

    # folded weights
    wqk = np.concatenate([q_w.T @ k_w, (q_w.T @ k_b)[:, None]], axis=1)  # (128,129)
    bqk = np.concatenate([q_b @ k_w, [q_b @ k_b]])[None, :]  # (1,129)
    wpack = np.ascontiguousarray(
        np.concatenate([wqk, v_w.T], axis=1, dtype=np.float32)
    )  # (128,257)
    bpack = np.ascontiguousarray(
        np.concatenate([bqk, v_b[None, :]], axis=1, dtype=np.float32)
    )  # (1,257)

    qf_flat = q_feat.reshape(B * N, C)
    gidx_flat = gidx.reshape(B * N, 9)

    in_maps = []
    for core in range(NCORES):
        lo = core * QPC
        hi = lo + QPC
        b = lo // N  # each core's queries live in a single batch
        assert (hi - 1) // N == b
        qf_c = np.ascontiguousarray(qf_flat[lo:hi])
        in_maps.append(
            {
                "featcl": fcl[b],
                "qpack": pack_queries(qf_c, gidx_flat[lo:hi]),
                "wpack": wpack,
                "bpack": bpack,
            }
        )
    return in_maps


_PROGRAM_CACHE = {}


def _get_program():
    key = (QPC, NVOX, N_ACT_MAC)
    if key not in _PROGRAM_CACHE:
        _PROGRAM_CACHE[key] = build_program(QPC, NVOX)
    return _PROGRAM_CACHE[key]


def run_on_hw(in_maps, trace=False, **kwargs):
    from concourse.bass_utils import run_bass_kernel_spmd

    nc = _get_program()
    return run_bass_kernel_spmd(nc, in_maps, list(range(NCORES)), trace=trace, **kwargs)


def kernel(q_feat, feat, proj_coord, hr_coord=None, q_w=None, q_b=None, k_w=None,
           k_b=None, v_w=None, v_b=None, **_unused):
    """Full inputs in, full output out. hr_coord is unused by the reference."""
    in_maps = host_prepare(q_feat, feat, proj_coord, q_w, q_b, k_w, k_b, v_w, v_b)
    res = run_on_hw(in_maps)
    parts = [res.results[c]["out"] for c in range(NCORES)]
    out = np.concatenate(parts, axis=0).reshape(B, N, C).astype(np.float32)
    return out



# revision 2
# speedup vs baseline: 1.0419x; 1.0419x over previous
"""Trainium2 Bass kernel v3 for nn_AttentionLayer_10591389352529 (sparse window attention).

Reference computation (per batch b, query n):
    local[n,k] = feat gathered at 3x3x3 window around proj_coord[n]   (k=0..26, c=128)
    q[n]       = q_feat[n] @ q_w.T + q_b
    score[n,k] = q[n] . (k_w @ local[n,k] + k_b)
    a[n,:]     = softmax(score[n,:])
    out[n]     = q_feat[n] + sum_k a[n,k] * (v_w @ local[n,k] + v_b)

Algebra:
    score[n,k] = qk[n] . local[n,k] + q[n].k_b   and the q.k_b term is
        constant over k -> cancels in softmax. qk = q_feat@(q_w.T@k_w) + q_b@k_w.
    out[n]     = q_feat[n] + v_w @ (sum_k a[n,k] local[n,k]) + v_b     (sum_k a = 1)
    v_b is folded into the shipped qft' = q_feat + v_b (used by the residual
    matmul); the qk bias is corrected to bqk' = q_b@k_w - v_b@(q_w.T@k_w) so
    qft'@wqk + bqk' still equals q_feat@wqk + q_b@k_w.

Wire format (per core, 3 buffers, all fp16 -> ~9.9 MB/core vs 59 MB in v1):
  featcl (33600,128) fp16: batch b = core//4's 14-plane D-slab (12 owned
      planes + 1 halo each side), channels-last, W edge-padded by 1.
  pack (128, 2886) fp16: [qft' (2304) | wqk (128) | vwt (128) |
      gidx int32 bits (324) | bqk' f32 bits (2)]
  out (2304,128) fp16 (donated zero buffer).
Queries are bucketed by d//12 on the host, padded to QPC=2304.
"""

import math
from contextlib import ExitStack

import numpy as np

import concourse.bass as bass
import concourse.tile as tile
from concourse import mybir
from concourse._compat import with_exitstack
from concourse.dve_ops import AFFINE_THEN_ADD, TENSOR_TENSOR_REDUCE
from concourse.tile_rust import add_dep_helper
import concourse.tile_sem_assignment as _tsa

_tsa.NUM_SWDGE_GLOBAL_SEMS = 1

FP32 = mybir.dt.float32
FP16 = mybir.dt.float16
INT32 = mybir.dt.int32
P = 128

B, N, C = 2, 8192, 128
D = H = W = 48
WP = W + 2
NSLAB = 4
SLAB_D = D // NSLAB  # 12
PLANES = SLAB_D + 2  # 14
SLAB_BASE = [0, 11, 23, 34]
NVOX = PLANES * H * WP  # 33600
NCORES = 8
QPC = 2304
KWIN = 27

N_ACT_MAC = 19

# pack column layout
COL_W = QPC  # wqk
COL_V = COL_W + P  # vwt
COL_G = COL_V + P  # gidx (324 fp16 cols = 162 int32); even offset for bitcast
COL_B = COL_G + 324  # bqk' (2 fp16 cols = 1 f32); even offset
PACK_COLS = COL_B + 2


@with_exitstack
def attention_body(
    ctx: ExitStack,
    tc: tile.TileContext,
    *,
    featcl: bass.AP,  # (nvox, 128) fp16
    pack: bass.AP,  # (128, PACK_COLS) fp16
    out: bass.AP,  # (qpc, 128) fp16
    n_act_mac: int = N_ACT_MAC,
):
    nc = tc.nc
    qpc = out.shape[0]
    ntiles = qpc // P
    assert qpc % (2 * P) == 0
    ngath = ntiles // 2
    GBUFS = 4
    ogroup = ntiles

    const = ctx.enter_context(tc.tile_pool(name="const", bufs=1))
    gath_pool = ctx.enter_context(tc.tile_pool(name="gath", bufs=GBUFS))
    qk_pool = ctx.enter_context(tc.tile_pool(name="qk", bufs=3))
    sc_pool = ctx.enter_context(tc.tile_pool(name="sc", bufs=ntiles))
    small_pool = ctx.enter_context(tc.tile_pool(name="small", bufs=ntiles))
    scratch_pool = ctx.enter_context(tc.tile_pool(name="scratch", bufs=4))
    mac_pool = ctx.enter_context(tc.tile_pool(name="mac", bufs=32))
    m_pool = ctx.enter_context(tc.tile_pool(name="m", bufs=4))
    stage_pool = ctx.enter_context(tc.tile_pool(name="stage", bufs=1))
    psum = ctx.enter_context(tc.tile_pool(name="psum", bufs=2, space="PSUM"))

    pack_sb = const.tile([P, PACK_COLS], FP16)
    nc.gpsimd.dma_start(pack_sb[:], pack[:, :])
    qft_full = pack_sb[:, 0:QPC]
    wqk_sb = pack_sb[:, COL_W : COL_W + P]
    vwt_sb = pack_sb[:, COL_V : COL_V + P]
    gidx_sb = pack_sb[:, COL_G : COL_G + 324].bitcast(INT32)  # [P, 162]
    bqk_col = pack_sb[:, COL_B : COL_B + 2].bitcast(FP32)  # [P, 1]

    ones_h = const.tile([1, P], FP16)
    nc.vector.memset(ones_h[:], 1.0)
    ident_f = const.tile([P, P], FP32)
    ident_h = const.tile([P, P], FP16)
    from concourse.masks import make_identity

    make_identity(nc, ident_f[:])
    make_identity(nc, ident_h[:])

    junk_pool = ctx.enter_context(tc.tile_pool(name="junk", bufs=8))

    # PE warmup: absorb every const-tensor dependency into PE's vector clock
    wu = psum.tile([P, P], FP32, space="PSUM", tag="qk_ps")
    nc.tensor.matmul(wu[:, 0:1], lhsT=ident_f[:], rhs=ident_f[:, 0:1], start=True, stop=True)
    nc.tensor.matmul(wu[:, 0:1], lhsT=ident_h[:], rhs=ident_h[:, 0:1], start=True, stop=True)
    for cst in (pack_sb[:], ones_h[:]):
        nc.tensor.matmul(
            wu[0:1, 0:1], lhsT=cst[:, 0:1], rhs=cst[:, 0:1], start=True, stop=True
        )

    # bqk' row: transpose the f32-bits column -> [1, P] then convert to fp16
    # (reuses the warmup PSUM tile's first partition row; no extra PSUM bank)
    bqkT_ps = psum.tile([P, P], FP32, space="PSUM", tag="qk_ps")
    nc.tensor.transpose(out=bqkT_ps[0:1, :], in_=bqk_col[:], identity=ident_f[:])
    bqk_sb = const.tile([1, P], FP16)
    nc.scalar.copy(bqk_sb[:], bqkT_ps[0:1, :])
    # absorb bqk_sb (ACT) into PE's clock so qk matmuls need no extra wait
    nc.tensor.matmul(
        wu[0:1, 0:1], lhsT=ones_h[:, 0:1], rhs=bqk_sb[0:1, 0:1], start=True, stop=True
    )

    stage = None
    sig_hist = []
    for t in range(ntiles):
        ts = bass.ts(t, P)
        g, tt = divmod(t, 2)

        if tt == 0:
            touches = []
            if g >= GBUFS:
                sg_prev = sig_hist[g - GBUFS]
                jtd = junk_pool.tile([1, 1], FP32, tag="junk_touch_d")
                touches.append(nc.gpsimd.tensor_copy(jtd[:], sg_prev[0:1, 0:1]))
                jta = junk_pool.tile([1, 1], FP32, tag="junk_touch_a")
                touches.append(nc.gpsimd.tensor_copy(jta[:], sg_prev[0:1, 1:2]))
            gath = [
                gath_pool.tile([P, 3 * P], FP16, tag=f"gath{j}", name=f"gath{j}_{g}")
                for j in range(18)
            ]
            for j in range(18):
                gi = nc.gpsimd.indirect_dma_start(
                    out=gath[j][:],
                    out_offset=None,
                    in_=featcl[:, :],
                    in_offset=bass.IndirectOffsetOnAxis(
                        ap=gidx_sb[:, g * 18 + j : g * 18 + j + 1], axis=0
                    ),
                )
                for tch in touches:
                    add_dep_helper(
                        gi.ins, tch.ins, sync=False, reason="gather after touches"
                    )

        def slab(k):
            j, v = tt * 9 + k // 3, k % 3
            return gath[j][:, v * P : (v + 1) * P]

        # ---- qk = qft' @ wqk + bqk'  (PE, fp16 inputs, f32 psum) ----
        qk_ps = psum.tile([P, P], FP32, space="PSUM", tag="qk_ps")
        nc.tensor.matmul(
            qk_ps[:], lhsT=qft_full[:, ts], rhs=wqk_sb[:], start=True, stop=False
        )
        nc.tensor.matmul(
            qk_ps[:], lhsT=ones_h[:], rhs=bqk_sb[:], start=False, stop=True
        )
        qk_sb = qk_pool.tile([P, P], FP32)
        nc.scalar.copy(qk_sb[:], qk_ps[:])

        # ---- scores[n,k] = qk[n] . slab_k[n]  (DVE fused reduce; the
        #      q.k_b bias is constant over k and cancels in softmax) ----
        scores = sc_pool.tile([P, KWIN], FP32, tag="scores")
        for k in range(KWIN):
            scr = scratch_pool.tile([P, P], FP32, tag="ttr_scr")
            nc.vector._custom_dve(
                TENSOR_TENSOR_REDUCE,
                out=scr[:],
                in0=qk_sb[:],
                in1=slab(k),
                s0=0.0,
                s1=1.0,
                accum_out=scores[:, k : k + 1],
            )

        # ---- softmax pieces ----
        negmax = small_pool.tile([P, 1], FP32, tag="negmax")
        nc.vector.tensor_reduce(
            out=negmax[:],
            in_=scores[:],
            axis=mybir.AxisListType.X,
            op=mybir.AluOpType.max,
            negate=True,
        )
        e = sc_pool.tile([P, KWIN], FP32, tag="e")
        sumexp = small_pool.tile([P, 1], FP32, tag="sumexp")
        nc.scalar.activation(
            e[:],
            scores[:],
            mybir.ActivationFunctionType.Exp,
            bias=negmax[:],
            scale=1.0,
            accum_out=sumexp[:],
        )
        recip = small_pool.tile([P, 1], FP32, tag="recip")
        nc.vector.reciprocal(recip[:], sumexp[:])
        jr = junk_pool.tile([1, 1], FP32, tag="junk_recip")
        nc.scalar.copy(jr[:], recip[0:1, :])

        # ---- m = sum_k e[:,k] * slab_k, accumulated in PSUM (f32) ----
        m_ps = psum.tile([P, P], FP32, space="PSUM", tag="m_ps")
        for k in range(n_act_mac):
            scaled = mac_pool.tile([P, P], FP32, tag="scaled")
            nc.scalar.mul(scaled[:], slab(k), e[:, k : k + 1])
            last_act_scaled = scaled
            nc.tensor.matmul(
                m_ps[:], lhsT=ident_f[:], rhs=scaled[:], start=(k == 0), stop=False
            )
        acc = mac_pool.tile([P, P], FP32, tag="dveacc")
        nc.vector.tensor_scalar_mul(acc[:], slab(n_act_mac), e[:, n_act_mac : n_act_mac + 1])
        for k in range(n_act_mac + 1, KWIN):
            acc2 = mac_pool.tile([P, P], FP32, tag="dveacc")
            nc.vector._custom_dve(
                AFFINE_THEN_ADD,
                out=acc2[:],
                in0=slab(k),
                in1=acc[:],
                s0=e[:, k : k + 1],
                s1=0.0,
            )
            acc = acc2
        last_dve_scaled = acc
        nc.tensor.matmul(m_ps[:], lhsT=ident_f[:], rhs=acc[:], start=False, stop=True)
        mr_sb = m_pool.tile([P, P], FP32, tag="mr_sb")
        nc.scalar.mul(mr_sb[:], m_ps[:], recip[:])

        # ---- x = mr @ v_w.T + (q_feat + v_b)  (PE) ----
        mt_ps = psum.tile([P, P], FP32, space="PSUM", tag="mt_ps")
        nc.tensor.transpose(out=mt_ps[:], in_=mr_sb[:], identity=ident_f[:])
        mt_sb = m_pool.tile([P, P], FP16, tag="mt_sb")
        nc.vector.tensor_copy(mt_sb[:], mt_ps[:])

        x_ps = psum.tile([P, P], FP32, space="PSUM", tag="x_ps")
        nc.tensor.matmul(x_ps[:], lhsT=mt_sb[:], rhs=vwt_sb[:], start=True, stop=False)
        # residual (+v_b): qft'_tile.T @ I
        nc.tensor.matmul(
            x_ps[:], lhsT=qft_full[:, ts], rhs=ident_h[:], start=False, stop=True
        )

        if t % ogroup == 0:
            stage = stage_pool.tile([P, ogroup * P], FP16, tag="stage")
        nc.scalar.copy(stage[:, bass.ts(t % ogroup, P)], x_ps[:])
        if t % ogroup == ogroup - 1:
            t0 = t - (ogroup - 1)
            jout = junk_pool.tile([1, 1], FP16, tag="junk_out")
            otch = nc.gpsimd.tensor_copy(jout[:], stage[0:1, ogroup * P - 1 :])
            od = nc.gpsimd.dma_start(
                out.rearrange("(t p) c -> p t c", p=P)[:, t0 : t0 + ogroup, :],
                stage[:].rearrange("p (t c) -> p t c", c=P),
            )
            add_dep_helper(od.ins, otch.ins, sync=False, reason="out after stage touch")

        if tt == 1:
            sg = junk_pool.tile([1, 2], FP32, tag="sig")
            nc.vector.tensor_copy(sg[0:1, 0:1], last_dve_scaled[0:1, 0:1])
            nc.scalar.copy(sg[0:1, 1:2], last_act_scaled[0:1, 0:1])
            sig_hist.append(sg)


def build_program(qpc: int, nvox: int, n_act_mac: int = N_ACT_MAC):
    nc = bass.Bass("TRN2", target_bir_lowering=False, debug=False, num_devices=NCORES)
    aps = {}
    decl = [
        ("featcl", (nvox, C), FP16, False),
        ("pack", (C, PACK_COLS), FP16, False),
        ("out", (qpc, C), FP16, True),
    ]
    for name, shape, dt, is_out in decl:
        kind = "ExternalOutput" if is_out else "ExternalInput"
        aps[name] = nc.dram_tensor(name, list(shape), dt, kind=kind).ap()
    with tile.TileContext(nc) as tc:
        attention_body(tc, n_act_mac=n_act_mac, **aps)
    mybir.codegen_inst_isa_subclasses(nc)
    _legalize_waits(nc)
    return nc


def _legalize_waits(nc, max_waits: int = 1):
    """Split >1-wait instructions: hoist extra waits onto preceding
    same-engine EventSemaphore instructions (engine queues are FIFO)."""
    for f in nc.m.functions:
        for blk in f.blocks:
            insts = blk.instructions
            i = 0
            while i < len(insts):
                inst = insts[i]
                si = inst.sync_info
                if si is not None and len(si.on_wait) > max_waits:
                    waits = list(si.on_wait)
                    pre = []
                    while len(waits) > max_waits:
                        chunk, waits = waits[:max_waits], waits[max_waits:]
                        pre.append(
                            mybir.InstEventSemaphore(
                                name=f"{inst.name}-ws{len(pre)}",
                                engine=inst.engine,
                                ins=[],
                                outs=[],
                                bass_nofuse=True,
                                sync_info=mybir.SyncInfo(on_wait=chunk, on_update=[]),
                            )
                        )
                    si.on_wait = waits
                    insts[i:i] = pre
                    i += len(pre)
                i += 1


def build_pack(qf_c: np.ndarray, gidx_c: np.ndarray, wqk, vwt, bqk) -> np.ndarray:
    """(128, PACK_COLS) fp16: [qft' | wqk | vwt | gidx bits | bqk' f32 bits]."""
    qpc = qf_c.shape[0]
    ntiles = qpc // P
    ngath = ntiles // 2
    qft = qf_c.T.astype(np.float16)  # (128, qpc)
    g = gidx_c.reshape(ngath, 2, P, 9)  # (g, tt, p, j)
    g = np.transpose(g, (2, 0, 1, 3)).reshape(P, ngath * 18)  # int32 (128,162)
    gbits = np.ascontiguousarray(g).view(np.float16)  # (128, 324)
    bbits = np.ascontiguousarray(bqk.astype(np.float32).reshape(P, 1)).view(np.float16)
    return np.ascontiguousarray(
        np.concatenate(
            [qft, wqk.astype(np.float16), vwt.astype(np.float16), gbits, bbits],
            axis=1,
        )
    )


def host_prepare(q_feat, feat, proj_coord, q_w, q_b, k_w, k_b, v_w, v_b):
    """Host-side marshalling. Returns (per-core input maps, per-core global
    query index lists for unsharding)."""
    q_feat = np.asarray(q_feat, dtype=np.float32)
    feat = np.asarray(feat, dtype=np.float32)
    proj_coord = np.asarray(proj_coord, dtype=np.int32)
    q_w, q_b, k_w, k_b, v_w, v_b = (
        np.asarray(a, dtype=np.float32) for a in (q_w, q_b, k_w, k_b, v_w, v_b)
    )

    # channels-last fp16 volume with W edge-padded by 1: (B, D, H, WP, C)
    fcl = np.transpose(feat, (0, 2, 3, 4, 1)).astype(np.float16)
    fcl = np.pad(fcl, ((0, 0), (0, 0), (0, 0), (1, 1), (0, 0)), mode="edge")

    d = proj_coord[..., 0].astype(np.int64)
    h = proj_coord[..., 1].astype(np.int64)
    w = proj_coord[..., 2].astype(np.int64)
    sid = np.minimum(d // SLAB_D, NSLAB - 1)

    wqk = q_w.T @ k_w  # (128,128)
    vwt = v_w.T  # (128,128)
    bqk = q_b @ k_w - v_b @ wqk  # (128,) corrected for the v_b fold into qft'

    offs = [(dd, hh) for dd in (-1, 0, 1) for hh in (-1, 0, 1)]
    in_maps = []
    sel_idx = []
    for core in range(NCORES):
        b, s = divmod(core, NSLAB)
        base = SLAB_BASE[s]
        sel = np.nonzero(sid[b] == s)[0]
        nq = sel.shape[0]
        assert nq <= QPC, f"core {core}: {nq} queries > QPC={QPC}"
        sel_idx.append(b * N + sel)

        qf_c = np.zeros((QPC, C), np.float32)
        qf_c[:nq] = q_feat[b, sel] + v_b  # v_b folded into the residual
        gidx = np.zeros((QPC, 9), np.int32)
        ds, hs, ws = d[b, sel], h[b, sel], w[b, sel]
        for j, (dd, hh) in enumerate(offs):
            dloc = np.clip(ds + dd, 0, D - 1) - base
            hc = np.clip(hs + hh, 0, H - 1)
            gidx[:nq, j] = ((dloc * H + hc) * WP + ws).astype(np.int32)

        featcl = np.ascontiguousarray(fcl[b, base : base + PLANES].reshape(NVOX, C))
        in_maps.append(
            {
                "featcl": featcl,
                "pack": build_pack(qf_c, gidx, wqk, vwt, bqk),
            }
        )
    return in_maps, sel_idx


_PROGRAM_CACHE = {}


def _get_program():
    key = (QPC, NVOX, N_ACT_MAC)
    if key not in _PROGRAM_CACHE:
        _PROGRAM_CACHE[key] = build_program(QPC, NVOX)
    return _PROGRAM_CACHE[key]


def run_on_hw(in_maps, trace=False, **kwargs):
    from concourse.bass_utils import run_bass_kernel_spmd

    nc = _get_program()
    return run_bass_kernel_spmd(nc, in_maps, list(range(NCORES)), trace=trace, **kwargs)


def unshard(per_core_outs, sel_idx):
    """Scatter per-core (QPC,C) fp16 outputs back to the full (B,N,C) f32."""
    full = np.empty((B * N, C), np.float32)
    for core in range(NCORES):
        idx = sel_idx[core]
        full[idx] = per_core_outs[core][: idx.shape[0]].astype(np.float32)
    return full.reshape(B, N, C)


def kernel(q_feat, feat, proj_coord, hr_coord=None, q_w=None, q_b=None, k_w=None,
           k_b=None, v_w=None, v_b=None, **_unused):
    """Full inputs in, full output out. hr_coord is unused by the reference."""
    in_maps, sel_idx = host_prepare(q_feat, feat, proj_coord, q_w, q_b, k_w, k_b,
                                    v_w, v_b)
    res = run_on_hw(in_maps)
    parts = [np.asarray(res.results[c]["out"]) for c in range(NCORES)]
    return unshard(parts, sel_idx)


# revision 3
# speedup vs baseline: 1.1798x; 1.1324x over previous
"""Trainium2 Bass kernel v3 for nn_AttentionLayer_10591389352529 (sparse window attention).

Reference computation (per batch b, query n):
    local[n,k] = feat gathered at 3x3x3 window around proj_coord[n]   (k=0..26, c=128)
    q[n]       = q_feat[n] @ q_w.T + q_b
    score[n,k] = q[n] . (k_w @ local[n,k] + k_b)
    a[n,:]     = softmax(score[n,:])
    out[n]     = q_feat[n] + sum_k a[n,k] * (v_w @ local[n,k] + v_b)

Algebra:
    score[n,k] = qk[n] . local[n,k] + q[n].k_b   and the q.k_b term is
        constant over k -> cancels in softmax. qk = q_feat@(q_w.T@k_w) + q_b@k_w.
    out[n]     = q_feat[n] + v_w @ (sum_k a[n,k] local[n,k]) + v_b     (sum_k a = 1)
    v_b is folded into the shipped qft' = q_feat + v_b (used by the residual
    matmul); the qk bias is corrected to bqk' = q_b@k_w - v_b@(q_w.T@k_w) so
    qft'@wqk + bqk' still equals q_feat@wqk + q_b@k_w.

Wire format (per core, 3 buffers, all fp16 -> ~9.3 MB/core vs 59 MB in v1):
  featcl (32256,128) fp16: batch b = core//4's 14-plane D-slab (12 owned
      planes + 1 halo each side), channels-last, W UNPADDED (48). The kernel
      expands it on device into an Internal padded volume (W=50, edge
      replicated) with three DMAs before the gathers start.
  pack (128, 2886) fp16: [qft' (2304) | wqk (128) | vwt (128) |
      gidx int32 bits (324) | bqk' f32 bits (2)]
  out (2304,128) fp16 (donated zero buffer).
Queries are bucketed by d//12 on the host, padded to QPC=2304.
"""

import math
from contextlib import ExitStack

import numpy as np

import concourse.bass as bass
import concourse.tile as tile
from concourse import mybir
from concourse._compat import with_exitstack
from concourse.dve_ops import AFFINE_THEN_ADD, TENSOR_TENSOR_REDUCE
from concourse.tile_rust import add_dep_helper
import concourse.tile_sem_assignment as _tsa

_tsa.NUM_SWDGE_GLOBAL_SEMS = 1

FP32 = mybir.dt.float32
FP16 = mybir.dt.float16
INT32 = mybir.dt.int32
P = 128

B, N, C = 2, 8192, 128
D = H = W = 48
WP = W + 2
NSLAB = 4
SLAB_D = D // NSLAB  # 12
PLANES = SLAB_D + 2  # 14
SLAB_BASE = [0, 11, 23, 34]
NVOX = PLANES * H * WP  # 33600 rows of the padded (on-device) volume
NVOX_IN = PLANES * H * W  # 32256 rows of the unpadded wire volume
NGRP = PLANES * H  # 672 (d,h) row-groups
NCORES = 8
QPC = 2304
KWIN = 27

N_ACT_MAC = 19

# pack column layout
COL_W = QPC  # wqk
COL_V = COL_W + P  # vwt
COL_G = COL_V + P  # gidx (324 fp16 cols = 162 int32); even offset for bitcast
COL_B = COL_G + 324  # bqk' (2 fp16 cols = 1 f32); even offset
PACK_COLS = COL_B + 2


@with_exitstack
def attention_body(
    ctx: ExitStack,
    tc: tile.TileContext,
    *,
    featin: bass.AP,  # (NVOX_IN, 128) fp16, W unpadded
    featcl: bass.AP,  # (NVOX, 128) fp16 Internal, W edge-padded by the kernel
    pack: bass.AP,  # (128, PACK_COLS) fp16
    out: bass.AP,  # (qpc, 128) fp16
    n_act_mac: int = N_ACT_MAC,
):
    nc = tc.nc
    qpc = out.shape[0]
    ntiles = qpc // P
    assert qpc % (2 * P) == 0
    ngath = ntiles // 2
    GBUFS = 4
    ogroup = ntiles

    const = ctx.enter_context(tc.tile_pool(name="const", bufs=1))
    gath_pool = ctx.enter_context(tc.tile_pool(name="gath", bufs=GBUFS))
    qk_pool = ctx.enter_context(tc.tile_pool(name="qk", bufs=3))
    sc_pool = ctx.enter_context(tc.tile_pool(name="sc", bufs=ntiles))
    small_pool = ctx.enter_context(tc.tile_pool(name="small", bufs=ntiles))
    scratch_pool = ctx.enter_context(tc.tile_pool(name="scratch", bufs=4))
    mac_pool = ctx.enter_context(tc.tile_pool(name="mac", bufs=32))
    m_pool = ctx.enter_context(tc.tile_pool(name="m", bufs=4))
    stage_pool = ctx.enter_context(tc.tile_pool(name="stage", bufs=1))
    psum = ctx.enter_context(tc.tile_pool(name="psum", bufs=2, space="PSUM"))

    # expand featin (W=48) -> featcl (W=50, edge-replicated) in DRAM:
    # bulk rows 1..48, then the two replicated edge rows
    src = featin.rearrange("(g r) c -> g r c", r=W)
    dst = featcl.rearrange("(g r) c -> g r c", r=WP)
    exp_dmas = [
        nc.gpsimd.dma_start(dst[:, 1 : W + 1, :], src[:, :, :]),
        nc.gpsimd.dma_start(dst[:, 0:1, :], src[:, 0:1, :]),
        nc.gpsimd.dma_start(dst[:, WP - 1 : WP, :], src[:, W - 1 : W, :]),
    ]

    pack_sb = const.tile([P, PACK_COLS], FP16)
    nc.gpsimd.dma_start(pack_sb[:], pack[:, :])
    qft_full = pack_sb[:, 0:QPC]
    wqk_sb = pack_sb[:, COL_W : COL_W + P]
    vwt_sb = pack_sb[:, COL_V : COL_V + P]
    gidx_sb = pack_sb[:, COL_G : COL_G + 324].bitcast(INT32)  # [P, 162]
    bqk_col = pack_sb[:, COL_B : COL_B + 2].bitcast(FP32)  # [P, 1]

    ones_h = const.tile([1, P], FP16)
    nc.vector.memset(ones_h[:], 1.0)
    ident_f = const.tile([P, P], FP32)
    ident_h = const.tile([P, P], FP16)
    from concourse.masks import make_identity

    make_identity(nc, ident_f[:])
    make_identity(nc, ident_h[:])

    junk_pool = ctx.enter_context(tc.tile_pool(name="junk", bufs=8))

    # PE warmup: absorb every const-tensor dependency into PE's vector clock
    wu = psum.tile([P, P], FP32, space="PSUM", tag="qk_ps")
    nc.tensor.matmul(wu[:, 0:1], lhsT=ident_f[:], rhs=ident_f[:, 0:1], start=True, stop=True)
    nc.tensor.matmul(wu[:, 0:1], lhsT=ident_h[:], rhs=ident_h[:, 0:1], start=True, stop=True)
    for cst in (pack_sb[:], ones_h[:]):
        nc.tensor.matmul(
            wu[0:1, 0:1], lhsT=cst[:, 0:1], rhs=cst[:, 0:1], start=True, stop=True
        )

    # bqk' row: transpose the f32-bits column -> [1, P] then convert to fp16
    # (reuses the warmup PSUM tile's first partition row; no extra PSUM bank)
    bqkT_ps = psum.tile([P, P], FP32, space="PSUM", tag="qk_ps")
    nc.tensor.transpose(out=bqkT_ps[0:1, :], in_=bqk_col[:], identity=ident_f[:])
    bqk_sb = const.tile([1, P], FP16)
    nc.scalar.copy(bqk_sb[:], bqkT_ps[0:1, :])
    # absorb bqk_sb (ACT) into PE's clock so qk matmuls need no extra wait
    nc.tensor.matmul(
        wu[0:1, 0:1], lhsT=ones_h[:, 0:1], rhs=bqk_sb[0:1, 0:1], start=True, stop=True
    )

    stage = None
    sig_hist = []
    for t in range(ntiles):
        ts = bass.ts(t, P)
        g, tt = divmod(t, 2)

        if tt == 0:
            touches = []
            if g >= GBUFS:
                sg_prev = sig_hist[g - GBUFS]
                jtd = junk_pool.tile([1, 1], FP32, tag="junk_touch_d")
                touches.append(nc.gpsimd.tensor_copy(jtd[:], sg_prev[0:1, 0:1]))
                jta = junk_pool.tile([1, 1], FP32, tag="junk_touch_a")
                touches.append(nc.gpsimd.tensor_copy(jta[:], sg_prev[0:1, 1:2]))
            gath = [
                gath_pool.tile([P, 3 * P], FP16, tag=f"gath{j}", name=f"gath{j}_{g}")
                for j in range(18)
            ]
            for j in range(18):
                gi = nc.gpsimd.indirect_dma_start(
                    out=gath[j][:],
                    out_offset=None,
                    in_=featcl[:, :],
                    in_offset=bass.IndirectOffsetOnAxis(
                        ap=gidx_sb[:, g * 18 + j : g * 18 + j + 1], axis=0
                    ),
                )
                for tch in touches:
                    add_dep_helper(
                        gi.ins, tch.ins, sync=False, reason="gather after touches"
                    )
                if g == 0:
                    # first gather period must observe the W-pad expansion
                    for ed in exp_dmas:
                        add_dep_helper(
                            gi.ins, ed.ins, sync=True, reason="gather after expand"
                        )

        def slab(k):
            j, v = tt * 9 + k // 3, k % 3
            return gath[j][:, v * P : (v + 1) * P]

        # ---- qk = qft' @ wqk + bqk'  (PE, fp16 inputs, f32 psum) ----
        qk_ps = psum.tile([P, P], FP32, space="PSUM", tag="qk_ps")
        nc.tensor.matmul(
            qk_ps[:], lhsT=qft_full[:, ts], rhs=wqk_sb[:], start=True, stop=False
        )
        nc.tensor.matmul(
            qk_ps[:], lhsT=ones_h[:], rhs=bqk_sb[:], start=False, stop=True
        )
        qk_sb = qk_pool.tile([P, P], FP32)
        nc.scalar.copy(qk_sb[:], qk_ps[:])

        # ---- scores[n,k] = qk[n] . slab_k[n]  (DVE fused reduce; the
        #      q.k_b bias is constant over k and cancels in softmax) ----
        scores = sc_pool.tile([P, KWIN], FP32, tag="scores")
        for k in range(KWIN):
            scr = scratch_pool.tile([P, P], FP32, tag="ttr_scr")
            nc.vector._custom_dve(
                TENSOR_TENSOR_REDUCE,
                out=scr[:],
                in0=qk_sb[:],
                in1=slab(k),
                s0=0.0,
                s1=1.0,
                accum_out=scores[:, k : k + 1],
            )

        # ---- softmax pieces ----
        negmax = small_pool.tile([P, 1], FP32, tag="negmax")
        nc.vector.tensor_reduce(
            out=negmax[:],
            in_=scores[:],
            axis=mybir.AxisListType.X,
            op=mybir.AluOpType.max,
            negate=True,
        )
        e = sc_pool.tile([P, KWIN], FP32, tag="e")
        sumexp = small_pool.tile([P, 1], FP32, tag="sumexp")
        nc.scalar.activation(
            e[:],
            scores[:],
            mybir.ActivationFunctionType.Exp,
            bias=negmax[:],
            scale=1.0,
            accum_out=sumexp[:],
        )
        recip = small_pool.tile([P, 1], FP32, tag="recip")
        nc.vector.reciprocal(recip[:], sumexp[:])
        jr = junk_pool.tile([1, 1], FP32, tag="junk_recip")
        nc.scalar.copy(jr[:], recip[0:1, :])

        # ---- m = sum_k e[:,k] * slab_k, accumulated in PSUM (f32) ----
        m_ps = psum.tile([P, P], FP32, space="PSUM", tag="m_ps")
        for k in range(n_act_mac):
            scaled = mac_pool.tile([P, P], FP32, tag="scaled")
            nc.scalar.mul(scaled[:], slab(k), e[:, k : k + 1])
            last_act_scaled = scaled
            nc.tensor.matmul(
                m_ps[:], lhsT=ident_f[:], rhs=scaled[:], start=(k == 0), stop=False
            )
        acc = mac_pool.tile([P, P], FP32, tag="dveacc")
        nc.vector.tensor_scalar_mul(acc[:], slab(n_act_mac), e[:, n_act_mac : n_act_mac + 1])
        for k in range(n_act_mac + 1, KWIN):
            acc2 = mac_pool.tile([P, P], FP32, tag="dveacc")
            nc.vector._custom_dve(
                AFFINE_THEN_ADD,
                out=acc2[:],
                in0=slab(k),
                in1=acc[:],
                s0=e[:, k : k + 1],
                s1=0.0,
            )
            acc = acc2
        last_dve_scaled = acc
        nc.tensor.matmul(m_ps[:], lhsT=ident_f[:], rhs=acc[:], start=False, stop=True)
        mr_sb = m_pool.tile([P, P], FP32, tag="mr_sb")
        nc.scalar.mul(mr_sb[:], m_ps[:], recip[:])

        # ---- x = mr @ v_w.T + (q_feat + v_b)  (PE) ----
        mt_ps = psum.tile([P, P], FP32, space="PSUM", tag="mt_ps")
        nc.tensor.transpose(out=mt_ps[:], in_=mr_sb[:], identity=ident_f[:])
        mt_sb = m_pool.tile([P, P], FP16, tag="mt_sb")
        nc.vector.tensor_copy(mt_sb[:], mt_ps[:])

        x_ps = psum.tile([P, P], FP32, space="PSUM", tag="x_ps")
        nc.tensor.matmul(x_ps[:], lhsT=mt_sb[:], rhs=vwt_sb[:], start=True, stop=False)
        # residual (+v_b): qft'_tile.T @ I
        nc.tensor.matmul(
            x_ps[:], lhsT=qft_full[:, ts], rhs=ident_h[:], start=False, stop=True
        )

        if t % ogroup == 0:
            stage = stage_pool.tile([P, ogroup * P], FP16, tag="stage")
        nc.scalar.copy(stage[:, bass.ts(t % ogroup, P)], x_ps[:])
        if t % ogroup == ogroup - 1:
            t0 = t - (ogroup - 1)
            jout = junk_pool.tile([1, 1], FP16, tag="junk_out")
            otch = nc.gpsimd.tensor_copy(jout[:], stage[0:1, ogroup * P - 1 :])
            od = nc.gpsimd.dma_start(
                out.rearrange("(t p) c -> p t c", p=P)[:, t0 : t0 + ogroup, :],
                stage[:].rearrange("p (t c) -> p t c", c=P),
            )
            add_dep_helper(od.ins, otch.ins, sync=False, reason="out after stage touch")

        if tt == 1:
            sg = junk_pool.tile([1, 2], FP32, tag="sig")
            nc.vector.tensor_copy(sg[0:1, 0:1], last_dve_scaled[0:1, 0:1])
            nc.scalar.copy(sg[0:1, 1:2], last_act_scaled[0:1, 0:1])
            sig_hist.append(sg)


def build_program(qpc: int, nvox: int, n_act_mac: int = N_ACT_MAC):
    nc = bass.Bass("TRN2", target_bir_lowering=False, debug=False, num_devices=NCORES)
    aps = {}
    decl = [
        ("featin", (NVOX_IN, C), FP16, "ExternalInput"),
        ("featcl", (nvox, C), FP16, "Internal"),
        ("pack", (C, PACK_COLS), FP16, "ExternalInput"),
        ("out", (qpc, C), FP16, "ExternalOutput"),
    ]
    for name, shape, dt, kind in decl:
        aps[name] = nc.dram_tensor(name, list(shape), dt, kind=kind).ap()
    with tile.TileContext(nc) as tc:
        attention_body(tc, n_act_mac=n_act_mac, **aps)
    mybir.codegen_inst_isa_subclasses(nc)
    _legalize_waits(nc)
    return nc


def _legalize_waits(nc, max_waits: int = 1):
    """Split >1-wait instructions: hoist extra waits onto preceding
    same-engine EventSemaphore instructions (engine queues are FIFO)."""
    for f in nc.m.functions:
        for blk in f.blocks:
            insts = blk.instructions
            i = 0
            while i < len(insts):
                inst = insts[i]
                si = inst.sync_info
                if si is not None and len(si.on_wait) > max_waits:
                    waits = list(si.on_wait)
                    pre = []
                    while len(waits) > max_waits:
                        chunk, waits = waits[:max_waits], waits[max_waits:]
                        pre.append(
                            mybir.InstEventSemaphore(
                                name=f"{inst.name}-ws{len(pre)}",
                                engine=inst.engine,
                                ins=[],
                                outs=[],
                                bass_nofuse=True,
                                sync_info=mybir.SyncInfo(on_wait=chunk, on_update=[]),
                            )
                        )
                    si.on_wait = waits
                    insts[i:i] = pre
                    i += len(pre)
                i += 1


def build_pack(qf_c: np.ndarray, gidx_c: np.ndarray, wqk, vwt, bqk) -> np.ndarray:
    """(128, PACK_COLS) fp16: [qft' | wqk | vwt | gidx bits | bqk' f32 bits]."""
    qpc = qf_c.shape[0]
    ntiles = qpc // P
    ngath = ntiles // 2
    qft = qf_c.T.astype(np.float16)  # (128, qpc)
    g = gidx_c.reshape(ngath, 2, P, 9)  # (g, tt, p, j)
    g = np.transpose(g, (2, 0, 1, 3)).reshape(P, ngath * 18)  # int32 (128,162)
    gbits = np.ascontiguousarray(g).view(np.float16)  # (128, 324)
    bbits = np.ascontiguousarray(bqk.astype(np.float32).reshape(P, 1)).view(np.float16)
    return np.ascontiguousarray(
        np.concatenate(
            [qft, wqk.astype(np.float16), vwt.astype(np.float16), gbits, bbits],
            axis=1,
        )
    )


def host_prepare(q_feat, feat, proj_coord, q_w, q_b, k_w, k_b, v_w, v_b):
    """Host-side marshalling. Returns (per-core input maps, per-core global
    query index lists for unsharding)."""
    q_feat = np.asarray(q_feat, dtype=np.float32)
    feat = np.asarray(feat, dtype=np.float32)
    proj_coord = np.asarray(proj_coord, dtype=np.int32)
    q_w, q_b, k_w, k_b, v_w, v_b = (
        np.asarray(a, dtype=np.float32) for a in (q_w, q_b, k_w, k_b, v_w, v_b)
    )

    # channels-last fp16 volume, W left unpadded (padded on device): (B,D,H,W,C)
    fcl = np.transpose(feat, (0, 2, 3, 4, 1)).astype(np.float16)

    d = proj_coord[..., 0].astype(np.int64)
    h = proj_coord[..., 1].astype(np.int64)
    w = proj_coord[..., 2].astype(np.int64)
    sid = np.minimum(d // SLAB_D, NSLAB - 1)

    wqk = q_w.T @ k_w  # (128,128)
    vwt = v_w.T  # (128,128)
    bqk = q_b @ k_w - v_b @ wqk  # (128,) corrected for the v_b fold into qft'

    offs = [(dd, hh) for dd in (-1, 0, 1) for hh in (-1, 0, 1)]
    in_maps = []
    sel_idx = []
    for core in range(NCORES):
        b, s = divmod(core, NSLAB)
        base = SLAB_BASE[s]
        sel = np.nonzero(sid[b] == s)[0]
        nq = sel.shape[0]
        assert nq <= QPC, f"core {core}: {nq} queries > QPC={QPC}"
        sel_idx.append(b * N + sel)

        qf_c = np.zeros((QPC, C), np.float32)
        qf_c[:nq] = q_feat[b, sel] + v_b  # v_b folded into the residual
        gidx = np.zeros((QPC, 9), np.int32)
        ds, hs, ws = d[b, sel], h[b, sel], w[b, sel]
        for j, (dd, hh) in enumerate(offs):
            dloc = np.clip(ds + dd, 0, D - 1) - base
            hc = np.clip(hs + hh, 0, H - 1)
            gidx[:nq, j] = ((dloc * H + hc) * WP + ws).astype(np.int32)

        featin = np.ascontiguousarray(fcl[b, base : base + PLANES].reshape(NVOX_IN, C))
        in_maps.append(
            {
                "featin": featin,
                "pack": build_pack(qf_c, gidx, wqk, vwt, bqk),
            }
        )
    return in_maps, sel_idx


_PROGRAM_CACHE = {}


def _get_program():
    key = (QPC, NVOX, N_ACT_MAC)
    if key not in _PROGRAM_CACHE:
        _PROGRAM_CACHE[key] = build_program(QPC, NVOX)
    return _PROGRAM_CACHE[key]


def run_on_hw(in_maps, trace=False, **kwargs):
    from concourse.bass_utils import run_bass_kernel_spmd

    nc = _get_program()
    return run_bass_kernel_spmd(nc, in_maps, list(range(NCORES)), trace=trace, **kwargs)


def unshard(per_core_outs, sel_idx):
    """Scatter per-core (QPC,C) fp16 outputs back to the full (B,N,C) f32."""
    full = np.empty((B * N, C), np.float32)
    for core in range(NCORES):
        idx = sel_idx[core]
        full[idx] = per_core_outs[core][: idx.shape[0]].astype(np.float32)
    return full.reshape(B, N, C)


def kernel(q_feat, feat, proj_coord, hr_coord=None, q_w=None, q_b=None, k_w=None,
           k_b=None, v_w=None, v_b=None, **_unused):
    """Full inputs in, full output out. hr_coord is unused by the reference."""
    in_maps, sel_idx = host_prepare(q_feat, feat, proj_coord, q_w, q_b, k_w, k_b,
                                    v_w, v_b)
    res = run_on_hw(in_maps)
    parts = [np.asarray(res.results[c]["out"]) for c in range(NCORES)]
    return unshard(parts, sel_idx)


# revision 4
# speedup vs baseline: 1050.9521x; 890.7597x over previous
"""Trainium2 Bass kernel v3 for nn_AttentionLayer_10591389352529 (sparse window attention).

Reference computation (per batch b, query n):
    local[n,k] = feat gathered at 3x3x3 window around proj_coord[n]   (k=0..26, c=128)
    q[n]       = q_feat[n] @ q_w.T + q_b
    score[n,k] = q[n] . (k_w @ local[n,k] + k_b)
    a[n,:]     = softmax(score[n,:])
    out[n]     = q_feat[n] + sum_k a[n,k] * (v_w @ local[n,k] + v_b)

Algebra:
    score[n,k] = qk[n] . local[n,k] + q[n].k_b   and the q.k_b term is
        constant over k -> cancels in softmax. qk = q_feat@(q_w.T@k_w) + q_b@k_w.
    out[n]     = q_feat[n] + v_w @ (sum_k a[n,k] local[n,k]) + v_b     (sum_k a = 1)
    v_b is folded into the shipped qft' = q_feat + v_b (used by the residual
    matmul); the qk bias is corrected to bqk' = q_b@k_w - v_b@(q_w.T@k_w) so
    qft'@wqk + bqk' still equals q_feat@wqk + q_b@k_w.

Wire format (per core, 3 buffers, all fp16 -> ~8.2 MB/core vs 59 MB in v1):
  featin (27648,128) fp16: batch b = core//4's 12 OWNED D-planes only (no
      halo), channels-last, W UNPADDED (48). Halos come from an on-device
      AllGather of every core-boundary plane within the batch's 4-core
      group; the kernel W-expands owned+exchanged planes into an Internal
      padded gather volume (20 planes x 48 x 50). The host-computed gather
      indices address owned planes directly and halo voxels inside the
      exchange region (which plane depends on the core - host knows).
  pack (128, 2886) fp16: [qft' (2304) | wqk (128) | vwt (128) |
      gidx int32 bits (324) | bqk' f32 bits (2)]
  out (2304,128) fp16 (donated zero buffer).
Queries are bucketed by d//12 on the host, padded to QPC=2304.
"""

import math
from contextlib import ExitStack

import numpy as np

import concourse.bass as bass
import concourse.tile as tile
from concourse import mybir
from concourse._compat import with_exitstack
from concourse.dve_ops import AFFINE_THEN_ADD, TENSOR_TENSOR_REDUCE
from concourse.tile_rust import add_dep_helper
import concourse.tile_sem_assignment as _tsa

_tsa.NUM_SWDGE_GLOBAL_SEMS = 1

FP32 = mybir.dt.float32
FP16 = mybir.dt.float16
INT32 = mybir.dt.int32
P = 128

B, N, C = 2, 8192, 128
D = H = W = 48
WP = W + 2
NSLAB = 4
SLAB_D = D // NSLAB  # 12 owned planes per core
NEX = 2 * NSLAB  # 8 exchanged boundary planes per batch
PLANES = SLAB_D + NEX  # 20 planes in the on-device gather volume
PLROWS = H * WP  # 2400 padded rows per plane
PLROWS_IN = H * W  # 2304 unpadded rows per plane
NVOX = PLANES * PLROWS  # 48000 rows of the padded (on-device) volume
NVOX_IN = SLAB_D * PLROWS_IN  # 27648 rows of the unpadded wire volume
NCORES = 8
QPC = 2304
KWIN = 27

N_ACT_MAC = 19

# pack column layout
COL_W = QPC  # wqk
COL_V = COL_W + P  # vwt
COL_G = COL_V + P  # gidx (324 fp16 cols = 162 int32); even offset for bitcast
COL_B = COL_G + 324  # bqk' (2 fp16 cols = 1 f32); even offset
PACK_COLS = COL_B + 2


@with_exitstack
def attention_body(
    ctx: ExitStack,
    tc: tile.TileContext,
    *,
    featin: bass.AP,  # (NVOX_IN, 128) fp16: 12 owned planes, W unpadded
    featcl: bass.AP,  # (NVOX, 128) fp16 Internal, W edge-padded by the kernel
    pack: bass.AP,  # (128, PACK_COLS) fp16
    out: bass.AP,  # (qpc, 128) fp16
    n_act_mac: int = N_ACT_MAC,
):
    nc = tc.nc
    qpc = out.shape[0]
    ntiles = qpc // P
    assert qpc % (2 * P) == 0
    ngath = ntiles // 2
    GBUFS = 4
    ogroup = ntiles

    const = ctx.enter_context(tc.tile_pool(name="const", bufs=1))
    gath_pool = ctx.enter_context(tc.tile_pool(name="gath", bufs=GBUFS))
    qk_pool = ctx.enter_context(tc.tile_pool(name="qk", bufs=3))
    sc_pool = ctx.enter_context(tc.tile_pool(name="sc", bufs=ntiles))
    small_pool = ctx.enter_context(tc.tile_pool(name="small", bufs=ntiles))
    scratch_pool = ctx.enter_context(tc.tile_pool(name="scratch", bufs=4))
    mac_pool = ctx.enter_context(tc.tile_pool(name="mac", bufs=32))
    m_pool = ctx.enter_context(tc.tile_pool(name="m", bufs=4))
    stage_pool = ctx.enter_context(tc.tile_pool(name="stage", bufs=1))
    psum = ctx.enter_context(tc.tile_pool(name="psum", bufs=2, space="PSUM"))

    # AllGather the two batch-group boundary planes of every core (planes 0
    # and 11 of featin) -> aggt = 8 planes ordered [first_r0, last_r0,
    # first_r1, last_r1, ...] by rank within the 4-core batch group.
    # Collectives need DRAM bounce tiles (not I/O tensors); the Tile
    # framework tracks deps through the pool tiles.
    dram = ctx.enter_context(tc.tile_pool(name="dram", bufs=1, space="DRAM"))
    bounce = dram.tile([2 * PLROWS_IN, C], FP16)
    aggt = dram.tile([NEX * PLROWS_IN, C], FP16)
    featin_pl = featin.rearrange("(p v) c -> p v c", v=PLROWS_IN)
    nc.gpsimd.dma_start(
        bounce[:].rearrange("(p v) c -> p v c", v=PLROWS_IN),
        featin_pl[0 : SLAB_D : SLAB_D - 1, :, :],
    )
    nc.gpsimd.collective_compute(
        kind="AllGather",
        op=mybir.AluOpType.bypass,
        replica_groups=[[0, 1, 2, 3], [4, 5, 6, 7]],
        ins=[bounce.opt()],
        outs=[aggt.opt()],
    )

    # expand (W=48) -> featcl (W=50, edge-replicated) in DRAM: 12 owned
    # planes from featin, then the 8 exchanged planes from aggt
    src = featin.rearrange("(g r) c -> g r c", r=W)
    dst = featcl.rearrange("(g r) c -> g r c", r=WP)
    own_g = SLAB_D * H  # (d,h) groups of the owned region
    agsrc = aggt[:].rearrange("(g r) c -> g r c", r=W)
    exp_dmas = [
        nc.gpsimd.dma_start(dst[:own_g, 1 : W + 1, :], src[:, :, :]),
        nc.gpsimd.dma_start(dst[:own_g, 0:1, :], src[:, 0:1, :]),
        nc.gpsimd.dma_start(dst[:own_g, WP - 1 : WP, :], src[:, W - 1 : W, :]),
        nc.gpsimd.dma_start(dst[own_g:, 1 : W + 1, :], agsrc[:, :, :]),
        nc.gpsimd.dma_start(dst[own_g:, 0:1, :], agsrc[:, 0:1, :]),
        nc.gpsimd.dma_start(dst[own_g:, WP - 1 : WP, :], agsrc[:, W - 1 : W, :]),
    ]

    pack_sb = const.tile([P, PACK_COLS], FP16)
    nc.gpsimd.dma_start(pack_sb[:], pack[:, :])
    qft_full = pack_sb[:, 0:QPC]
    wqk_sb = pack_sb[:, COL_W : COL_W + P]
    vwt_sb = pack_sb[:, COL_V : COL_V + P]
    gidx_sb = pack_sb[:, COL_G : COL_G + 324].bitcast(INT32)  # [P, 162]
    bqk_col = pack_sb[:, COL_B : COL_B + 2].bitcast(FP32)  # [P, 1]

    ones_h = const.tile([1, P], FP16)
    nc.vector.memset(ones_h[:], 1.0)
    ident_f = const.tile([P, P], FP32)
    ident_h = const.tile([P, P], FP16)
    from concourse.masks import make_identity

    make_identity(nc, ident_f[:])
    make_identity(nc, ident_h[:])

    junk_pool = ctx.enter_context(tc.tile_pool(name="junk", bufs=8))

    # PE warmup: absorb every const-tensor dependency into PE's vector clock
    wu = psum.tile([P, P], FP32, space="PSUM", tag="qk_ps")
    nc.tensor.matmul(wu[:, 0:1], lhsT=ident_f[:], rhs=ident_f[:, 0:1], start=True, stop=True)
    nc.tensor.matmul(wu[:, 0:1], lhsT=ident_h[:], rhs=ident_h[:, 0:1], start=True, stop=True)
    for cst in (pack_sb[:], ones_h[:]):
        nc.tensor.matmul(
            wu[0:1, 0:1], lhsT=cst[:, 0:1], rhs=cst[:, 0:1], start=True, stop=True
        )

    # bqk' row: transpose the f32-bits column -> [1, P] then convert to fp16
    # (reuses the warmup PSUM tile's first partition row; no extra PSUM bank)
    bqkT_ps = psum.tile([P, P], FP32, space="PSUM", tag="qk_ps")
    nc.tensor.transpose(out=bqkT_ps[0:1, :], in_=bqk_col[:], identity=ident_f[:])
    bqk_sb = const.tile([1, P], FP16)
    nc.scalar.copy(bqk_sb[:], bqkT_ps[0:1, :])
    # absorb bqk_sb (ACT) into PE's clock so qk matmuls need no extra wait
    nc.tensor.matmul(
        wu[0:1, 0:1], lhsT=ones_h[:, 0:1], rhs=bqk_sb[0:1, 0:1], start=True, stop=True
    )

    stage = None
    sig_hist = []
    for t in range(ntiles):
        ts = bass.ts(t, P)
        g, tt = divmod(t, 2)

        if tt == 0:
            touches = []
            if g >= GBUFS:
                sg_prev = sig_hist[g - GBUFS]
                jtd = junk_pool.tile([1, 1], FP32, tag="junk_touch_d")
                touches.append(nc.gpsimd.tensor_copy(jtd[:], sg_prev[0:1, 0:1]))
                jta = junk_pool.tile([1, 1], FP32, tag="junk_touch_a")
                touches.append(nc.gpsimd.tensor_copy(jta[:], sg_prev[0:1, 1:2]))
            gath = [
                gath_pool.tile([P, 3 * P], FP16, tag=f"gath{j}", name=f"gath{j}_{g}")
                for j in range(18)
            ]
            for j in range(18):
                gi = nc.gpsimd.indirect_dma_start(
                    out=gath[j][:],
                    out_offset=None,
                    in_=featcl[:, :],
                    in_offset=bass.IndirectOffsetOnAxis(
                        ap=gidx_sb[:, g * 18 + j : g * 18 + j + 1], axis=0
                    ),
                )
                for tch in touches:
                    add_dep_helper(
                        gi.ins, tch.ins, sync=False, reason="gather after touches"
                    )
                if g == 0:
                    # first gather period must observe the W-pad expansion
                    for ed in exp_dmas:
                        add_dep_helper(
                            gi.ins, ed.ins, sync=True, reason="gather after expand"
                        )

        def slab(k):
            j, v = tt * 9 + k // 3, k % 3
            return gath[j][:, v * P : (v + 1) * P]

        # ---- qk = qft' @ wqk + bqk'  (PE, fp16 inputs, f32 psum) ----
        qk_ps = psum.tile([P, P], FP32, space="PSUM", tag="qk_ps")
        nc.tensor.matmul(
            qk_ps[:], lhsT=qft_full[:, ts], rhs=wqk_sb[:], start=True, stop=False
        )
        nc.tensor.matmul(
            qk_ps[:], lhsT=ones_h[:], rhs=bqk_sb[:], start=False, stop=True
        )
        qk_sb = qk_pool.tile([P, P], FP32)
        nc.scalar.copy(qk_sb[:], qk_ps[:])

        # ---- scores[n,k] = qk[n] . slab_k[n]  (DVE fused reduce; the
        #      q.k_b bias is constant over k and cancels in softmax) ----
        scores = sc_pool.tile([P, KWIN], FP32, tag="scores")
        for k in range(KWIN):
            scr = scratch_pool.tile([P, P], FP32, tag="ttr_scr")
            nc.vector._custom_dve(
                TENSOR_TENSOR_REDUCE,
                out=scr[:],
                in0=qk_sb[:],
                in1=slab(k),
                s0=0.0,
                s1=1.0,
                accum_out=scores[:, k : k + 1],
            )

        # ---- softmax pieces ----
        negmax = small_pool.tile([P, 1], FP32, tag="negmax")
        nc.vector.tensor_reduce(
            out=negmax[:],
            in_=scores[:],
            axis=mybir.AxisListType.X,
            op=mybir.AluOpType.max,
            negate=True,
        )
        e = sc_pool.tile([P, KWIN], FP32, tag="e")
        sumexp = small_pool.tile([P, 1], FP32, tag="sumexp")
        nc.scalar.activation(
            e[:],
            scores[:],
            mybir.ActivationFunctionType.Exp,
            bias=negmax[:],
            scale=1.0,
            accum_out=sumexp[:],
        )
        recip = small_pool.tile([P, 1], FP32, tag="recip")
        nc.vector.reciprocal(recip[:], sumexp[:])
        jr = junk_pool.tile([1, 1], FP32, tag="junk_recip")
        nc.scalar.copy(jr[:], recip[0:1, :])

        # ---- m = sum_k e[:,k] * slab_k, accumulated in PSUM (f32) ----
        m_ps = psum.tile([P, P], FP32, space="PSUM", tag="m_ps")
        for k in range(n_act_mac):
            scaled = mac_pool.tile([P, P], FP32, tag="scaled")
            nc.scalar.mul(scaled[:], slab(k), e[:, k : k + 1])
            last_act_scaled = scaled
            nc.tensor.matmul(
                m_ps[:], lhsT=ident_f[:], rhs=scaled[:], start=(k == 0), stop=False
            )
        acc = mac_pool.tile([P, P], FP32, tag="dveacc")
        nc.vector.tensor_scalar_mul(acc[:], slab(n_act_mac), e[:, n_act_mac : n_act_mac + 1])
        for k in range(n_act_mac + 1, KWIN):
            acc2 = mac_pool.tile([P, P], FP32, tag="dveacc")
            nc.vector._custom_dve(
                AFFINE_THEN_ADD,
                out=acc2[:],
                in0=slab(k),
                in1=acc[:],
                s0=e[:, k : k + 1],
                s1=0.0,
            )
            acc = acc2
        last_dve_scaled = acc
        nc.tensor.matmul(m_ps[:], lhsT=ident_f[:], rhs=acc[:], start=False, stop=True)
        mr_sb = m_pool.tile([P, P], FP32, tag="mr_sb")
        nc.scalar.mul(mr_sb[:], m_ps[:], recip[:])

        # ---- x = mr @ v_w.T + (q_feat + v_b)  (PE) ----
        mt_ps = psum.tile([P, P], FP32, space="PSUM", tag="mt_ps")
        nc.tensor.transpose(out=mt_ps[:], in_=mr_sb[:], identity=ident_f[:])
        mt_sb = m_pool.tile([P, P], FP16, tag="mt_sb")
        nc.vector.tensor_copy(mt_sb[:], mt_ps[:])

        x_ps = psum.tile([P, P], FP32, space="PSUM", tag="x_ps")
        nc.tensor.matmul(x_ps[:], lhsT=mt_sb[:], rhs=vwt_sb[:], start=True, stop=False)
        # residual (+v_b): qft'_tile.T @ I
        nc.tensor.matmul(
            x_ps[:], lhsT=qft_full[:, ts], rhs=ident_h[:], start=False, stop=True
        )

        if t % ogroup == 0:
            stage = stage_pool.tile([P, ogroup * P], FP16, tag="stage")
        nc.scalar.copy(stage[:, bass.ts(t % ogroup, P)], x_ps[:])
        if t % ogroup == ogroup - 1:
            t0 = t - (ogroup - 1)
            jout = junk_pool.tile([1, 1], FP16, tag="junk_out")
            otch = nc.gpsimd.tensor_copy(jout[:], stage[0:1, ogroup * P - 1 :])
            od = nc.gpsimd.dma_start(
                out.rearrange("(t p) c -> p t c", p=P)[:, t0 : t0 + ogroup, :],
                stage[:].rearrange("p (t c) -> p t c", c=P),
            )
            add_dep_helper(od.ins, otch.ins, sync=False, reason="out after stage touch")

        if tt == 1:
            sg = junk_pool.tile([1, 2], FP32, tag="sig")
            nc.vector.tensor_copy(sg[0:1, 0:1], last_dve_scaled[0:1, 0:1])
            nc.scalar.copy(sg[0:1, 1:2], last_act_scaled[0:1, 0:1])
            sig_hist.append(sg)


def build_program(qpc: int, nvox: int, n_act_mac: int = N_ACT_MAC):
    nc = bass.Bass("TRN2", target_bir_lowering=False, debug=False, num_devices=NCORES)
    aps = {}
    decl = [
        ("featin", (NVOX_IN, C), FP16, "ExternalInput"),
        ("featcl", (nvox, C), FP16, "Internal"),
        ("pack", (C, PACK_COLS), FP16, "ExternalInput"),
        ("out", (qpc, C), FP16, "ExternalOutput"),
    ]
    for name, shape, dt, kind in decl:
        aps[name] = nc.dram_tensor(name, list(shape), dt, kind=kind).ap()
    with tile.TileContext(nc) as tc:
        attention_body(tc, n_act_mac=n_act_mac, **aps)
    mybir.codegen_inst_isa_subclasses(nc)
    _legalize_waits(nc)
    return nc


def _legalize_waits(nc, max_waits: int = 1):
    """Split >1-wait instructions: hoist extra waits onto preceding
    same-engine EventSemaphore instructions (engine queues are FIFO)."""
    for f in nc.m.functions:
        for blk in f.blocks:
            insts = blk.instructions
            i = 0
            while i < len(insts):
                inst = insts[i]
                si = inst.sync_info
                if si is not None and len(si.on_wait) > max_waits:
                    waits = list(si.on_wait)
                    pre = []
                    while len(waits) > max_waits:
                        chunk, waits = waits[:max_waits], waits[max_waits:]
                        pre.append(
                            mybir.InstEventSemaphore(
                                name=f"{inst.name}-ws{len(pre)}",
                                engine=inst.engine,
                                ins=[],
                                outs=[],
                                bass_nofuse=True,
                                sync_info=mybir.SyncInfo(on_wait=chunk, on_update=[]),
                            )
                        )
                    si.on_wait = waits
                    insts[i:i] = pre
                    i += len(pre)
                i += 1


def build_pack(qf_c: np.ndarray, gidx_c: np.ndarray, wqk, vwt, bqk) -> np.ndarray:
    """(128, PACK_COLS) fp16: [qft' | wqk | vwt | gidx bits | bqk' f32 bits]."""
    qpc = qf_c.shape[0]
    ntiles = qpc // P
    ngath = ntiles // 2
    qft = qf_c.T.astype(np.float16)  # (128, qpc)
    g = gidx_c.reshape(ngath, 2, P, 9)  # (g, tt, p, j)
    g = np.transpose(g, (2, 0, 1, 3)).reshape(P, ngath * 18)  # int32 (128,162)
    gbits = np.ascontiguousarray(g).view(np.float16)  # (128, 324)
    bbits = np.ascontiguousarray(bqk.astype(np.float32).reshape(P, 1)).view(np.float16)
    return np.ascontiguousarray(
        np.concatenate(
            [qft, wqk.astype(np.float16), vwt.astype(np.float16), gbits, bbits],
            axis=1,
        )
    )


def host_prepare(q_feat, feat, proj_coord, q_w, q_b, k_w, k_b, v_w, v_b):
    """Host-side marshalling. Returns (per-core input maps, per-core global
    query index lists for unsharding)."""
    q_feat = np.asarray(q_feat, dtype=np.float32)
    feat = np.asarray(feat, dtype=np.float32)
    proj_coord = np.asarray(proj_coord, dtype=np.int32)
    q_w, q_b, k_w, k_b, v_w, v_b = (
        np.asarray(a, dtype=np.float32) for a in (q_w, q_b, k_w, k_b, v_w, v_b)
    )

    # channels-last fp16 volume, W left unpadded (padded on device): (B,D,H,W,C)
    fcl = np.transpose(feat, (0, 2, 3, 4, 1)).astype(np.float16)

    d = proj_coord[..., 0].astype(np.int64)
    h = proj_coord[..., 1].astype(np.int64)
    w = proj_coord[..., 2].astype(np.int64)
    sid = np.minimum(d // SLAB_D, NSLAB - 1)  # d < 48 so already in [0,3]

    wqk = q_w.T @ k_w  # (128,128)
    vwt = v_w.T  # (128,128)
    bqk = q_b @ k_w - v_b @ wqk  # (128,) corrected for the v_b fold into qft'

    offs = [(dd, hh) for dd in (-1, 0, 1) for hh in (-1, 0, 1)]
    in_maps = []
    sel_idx = []
    for core in range(NCORES):
        b, s = divmod(core, NSLAB)
        base = s * SLAB_D
        sel = np.nonzero(sid[b] == s)[0]
        nq = sel.shape[0]
        assert nq <= QPC, f"core {core}: {nq} queries > QPC={QPC}"
        sel_idx.append(b * N + sel)

        qf_c = np.zeros((QPC, C), np.float32)
        qf_c[:nq] = q_feat[b, sel] + v_b  # v_b folded into the residual
        gidx = np.zeros((QPC, 9), np.int32)
        ds, hs, ws = d[b, sel], h[b, sel], w[b, sel]
        for j, (dd, hh) in enumerate(offs):
            g = np.clip(ds + dd, 0, D - 1)
            # owned plane -> local plane g-base; neighbor boundary plane ->
            # exchange plane SLAB_D + 2*(g//12) + (1 if g%12==11 else 0)
            own = (g >= base) & (g < base + SLAB_D)
            pl = np.where(
                own, g - base, SLAB_D + 2 * (g // SLAB_D) + (g % SLAB_D == SLAB_D - 1)
            )
            hc = np.clip(hs + hh, 0, H - 1)
            gidx[:nq, j] = (pl * PLROWS + hc * WP + ws).astype(np.int32)

        featin = np.ascontiguousarray(
            fcl[b, base : base + SLAB_D].reshape(NVOX_IN, C)
        )
        in_maps.append(
            {
                "featin": featin,
                "pack": build_pack(qf_c, gidx, wqk, vwt, bqk),
            }
        )
    return in_maps, sel_idx


_PROGRAM_CACHE = {}


def _get_program():
    key = (QPC, NVOX, N_ACT_MAC)
    if key not in _PROGRAM_CACHE:
        _PROGRAM_CACHE[key] = build_program(QPC, NVOX)
    return _PROGRAM_CACHE[key]


def run_on_hw(in_maps, trace=False, **kwargs):
    from concourse.bass_utils import run_bass_kernel_spmd

    nc = _get_program()
    return run_bass_kernel_spmd(nc, in_maps, list(range(NCORES)), trace=trace, **kwargs)


def unshard(per_core_outs, sel_idx):
    """Scatter per-core (QPC,C) fp16 outputs back to the full (B,N,C) f32."""
    full = np.empty((B * N, C), np.float32)
    for core in range(NCORES):
        idx = sel_idx[core]
        full[idx] = per_core_outs[core][: idx.shape[0]].astype(np.float32)
    return full.reshape(B, N, C)


def kernel(q_feat, feat, proj_coord, hr_coord=None, q_w=None, q_b=None, k_w=None,
           k_b=None, v_w=None, v_b=None, **_unused):
    """Full inputs in, full output out. hr_coord is unused by the reference."""
    in_maps, sel_idx = host_prepare(q_feat, feat, proj_coord, q_w, q_b, k_w, k_b,
                                    v_w, v_b)
    res = run_on_hw(in_maps)
    parts = [np.asarray(res.results[c]["out"]) for c in range(NCORES)]
    return unshard(parts, sel_idx)
